# revision 31
# baseline (speedup 1.0000x reference)
"""DecodeBox (3D YOLO-style box decode) Trainium2 Bass kernel.

Input : inp [16, 18, 48, 48, 48] f32  (= [B, A*ATTRS, D, H, W], A=3, ATTRS=6)
Output: out [16, 331776, 6] f32       (= [B, A*D*H*W, (bx,by,bz,bl,conf,cls)])

Math (per anchor a, spatial cell s=(zd,y,x), channel layout c in 0..5):
  bx = (sigmoid(v0) + gx) * 2      gx = x
  by = (sigmoid(v1) + gy) * 2      gy = y
  bz = (sigmoid(v2) + gz) * 2      gz = zd
  bl = exp(v3) * anchor_w[a]       anchor_w = (4, 8, 16)
  conf = sigmoid(v4)
  cls  = sigmoid(v5)

Sharding: batch dim across 8 cores (2 batches per core), no communication.

Per-core layout strategy: for each (b, a) block the input is [6, 110592]
channel-major while the output needs [110592, 6] attr-interleaved. Each
block is one DMA into an SBUF tile [128, 6, 864] (partition p holds spatial
positions p*864..p*864+863 of each channel); ACT computes tanh/exp (all in
one activation table set, using sigmoid(v) == 0.5*tanh(v/2)+0.5) and DVE
applies the grid/affine terms, writing into an interleaved [128, 864, 6]
tile that one contiguous DMA stores. Grid addends live in a tiny [128, 87]
constant table read through stride-0 broadcast APs. Loads are issued from
the Sync HWDGE ring and stores from the GpSimd SWDGE ring so compute-gated
stores never block later loads.

Scheduling (the part that matters for perf): the kernel is DMA-bound
(~16 MB in + ~16 MB out per core; the 16 DMA engines stream 3456 B load
descriptors at ~25 GB/s each and 20736 B store descriptors at ~27-30).
The engines round-robin per DESCRIPTOR between their HW (load) and SW
(store) queues, so once stores are queued they take ~6x the bandwidth and
starve pending loads, which stalls compute and collapses the pipeline
(trace: block loads triggered at 9us not completing until 42-73us). All
loads are therefore issued up front, and each block's store is data-gated
on the load TWO blocks ahead via a benign 1-element bypass op that reads
that input tile: when loads flow freely the gate never delays anything
(the overlapped schedule keeps all engines busy), and when stores start
winning the descriptor round-robin the gate pauses new stores until loads
catch back up. Descriptor sizes are load-layout-optimal already: bigger
(6912 B) SBUF-writing descriptors measured 2x SLOWER per byte, and
partition counts that aren't multiples of 16 scramble the partition->
engine banking (3x slower) - don't touch the 128x(p,c) descriptor shape.
"""

import sys

if "/opt/trn_rl_repo" not in sys.path:
    sys.path.insert(0, "/opt/trn_rl_repo")

import numpy as np

import concourse.bacc as bacc
import concourse.bass as bass
import concourse.mybir as mybir
from concourse.bass_utils import run_bass_kernel_spmd
from concourse.tile import TileContext

B = 16
A = 3
ATTRS = 6
G = 48                # grid size per axis
S = G * G * G         # 110592 spatial positions
N_CORES = 8
B_LOC = B // N_CORES  # 2 batches per core
P = 128               # SBUF partitions
FREE = S // P         # 864 spatial positions per partition
STRIDE = 2.0          # IMG_SIZE / grid = 96 / 48
ANCHOR_W = (4.0, 8.0, 16.0)

_NC = None
last_results = None  # BassKernelResults of the most recent run (for profiling)
trace = False        # set True before calling kernel() to capture an NTFF trace


YZ = FREE // G  # 18 (y,z)-rows per partition

# Pair-merged layout: one SBUF tile covers TWO (b, a) blocks; partitions
# 0..63 hold block 2q ("X"), partitions 64..127 hold block 2q+1 ("Y"),
# each partition covering 1728 consecutive spatial positions of its block.
PH = 64                 # partitions per block in a pair tile
FREE2 = S // PH         # 1728 spatial positions per partition
YZ2 = FREE2 // G        # 36 (y,z)-rows per partition
N_PAIR = (B_LOC * A) // 2  # 3 pair tiles per core


def _consts2() -> np.ndarray:
    """[128, 48+36+36+3] f32 constant table for the pair-merged kernel.

    (sigmoid(v) + g)*2 == tanh(v/2) + (2g + 1); s = (p%64)*1728 + r*48 + x.

      [:, 0:48]     2*x + 1                 (same for every partition)
      [:, 48:84]    2*y + 1  per (p%64, r)
      [:, 84:120]   2*z + 1  per (p%64, r)
      [:, 120:123]  per-pair exp bias: ln(anchor_w[blk%3]) where
                    blk = 2*pair + (p >= 64)
    """
    t = np.empty((P, G + 2 * YZ2 + N_PAIR), dtype=np.float32)
    x = np.arange(G, dtype=np.float32)
    pp = (np.arange(P, dtype=np.int64) % PH)[:, None]
    rows = pp * YZ2 + np.arange(YZ2)[None, :]
    t[:, 0:G] = x * STRIDE + 1.0
    t[:, G : G + YZ2] = (rows % G) * STRIDE + 1.0
    t[:, G + YZ2 : G + 2 * YZ2] = (rows // G) * STRIDE + 1.0
    lw = np.log(np.array(ANCHOR_W, dtype=np.float32))
    for q in range(N_PAIR):
        t[:PH, G + 2 * YZ2 + q] = lw[(2 * q) % A]
        t[PH:, G + 2 * YZ2 + q] = lw[(2 * q + 1) % A]
    return t


def _build_pairs(
    io_bufs: int = 2,
    out_bufs: int = 2,
    tmp_bufs: int = 2,
    sig_engine: str = "vector",
    gate_stores: bool = True,
    gate_engine: str = "gpsimd",
) -> bass.Bass:
    """Pair-merged build: 3 tiles of 2 blocks each.

    Wins over the per-block build:
      - load descriptors are 6912 B (vs 3456) -> less per-descriptor DMA
        engine overhead on the 16 HW queues;
      - each pair's output is ONE flat SWDGE store DMA with a single
        41472 B descriptor per partition (the two blocks are contiguous in
        DRAM and the partition->address map is affine), so 3 store DMAs
        total at near-streaming descriptor efficiency.
    """
    nc = bacc.Bacc("TRN2", target_bir_lowering=False, debug=False)
    inp = nc.dram_tensor(
        "inp", [B_LOC, A * ATTRS, G, G, G], mybir.dt.float32, kind="ExternalInput"
    )
    consts = nc.dram_tensor(
        "consts", [P, G + 2 * YZ2 + N_PAIR], mybir.dt.float32, kind="ExternalInput"
    )
    out = nc.dram_tensor(
        "out", [B_LOC, A * S, ATTRS], mybir.dt.float32, kind="ExternalOutput"
    )

    # [6 blocks, 6 ch, S] -> per pair q: [(2 blocks * 64 p) = 128, 6, 1728]
    inp_r = inp.ap().rearrange("b (a c) d h w -> (b a) c (d h w)", a=A)
    # all 6 blocks contiguous in DRAM: [(b a p) = 384, (j k) = 10368]
    out_r = out.ap().rearrange("b (a p j) k -> (b a p) (j k)", a=A, p=PH)

    F = mybir.ActivationFunctionType
    Op = mybir.AluOpType
    f32 = mybir.dt.float32

    with TileContext(nc) as tc:
        with (
            tc.tile_pool(name="const", bufs=1) as cpool,
            tc.tile_pool(name="io", bufs=io_bufs) as iopool,
            tc.tile_pool(name="io_out", bufs=out_bufs) as opool,
            tc.tile_pool(name="tmp", bufs=tmp_bufs) as tpool,
        ):
            ct = cpool.tile([P, G + 2 * YZ2 + N_PAIR], f32)
            nc.sync.dma_start(out=ct[:], in_=consts.ap())
            lw = ct[:, G + 2 * YZ2 :]
            sig_eng = getattr(nc, sig_engine)
            gate_eng = getattr(nc, gate_engine)

            grids = (
                ct[:, 0:G].unsqueeze(1).broadcast_to([P, YZ2, G]),
                ct[:, G : G + YZ2].unsqueeze(2).broadcast_to([P, YZ2, G]),
                ct[:, G + YZ2 : G + 2 * YZ2].unsqueeze(2).broadcast_to([P, YZ2, G]),
            )

            # all loads up front (see _build docstring: stores are gated on
            # the last load so loads stream with zero store interference)
            xs = []
            for q in range(N_PAIR):
                x = iopool.tile([P, ATTRS, FREE2], f32, tag="in")
                for h in range(2):
                    nc.sync.dma_start(
                        out=x[h * PH : (h + 1) * PH],
                        in_=inp_r[2 * q + h].rearrange("c (p j) -> p c j", p=PH),
                    )
                xs.append(x)
            x_last = xs[-1]

            for q in range(N_PAIR):
                x = xs[q]
                o = opool.tile([P, FREE2, ATTRS], f32, tag="out")
                # ch 0-2: one merged tanh ACT, then per-channel grid adds
                t3 = tpool.tile([P, 3, FREE2], f32, tag="t")
                nc.scalar.activation(t3[:], x[:, 0:3, :], F.Tanh, scale=0.5)
                for c in range(3):
                    nc.vector.tensor_add(
                        o[:, :, c].rearrange("p (r g) -> p r g", g=G),
                        t3[:, c, :].rearrange("p (r g) -> p r g", g=G),
                        grids[c],
                    )
                # ch 3: exp(v + ln(anchor_w)), per-partition bias table
                nc.scalar.activation(
                    o[:, :, 3], x[:, 3, :], F.Exp, bias=lw[:, q : q + 1]
                )
                # ch 4-5: one merged tanh ACT -> tmp, then affine interleave
                t45 = tpool.tile([P, 2, FREE2], f32, tag="t")
                nc.scalar.activation(t45[:], x[:, 4:6, :], F.Tanh, scale=0.5)
                sig_eng.tensor_scalar(
                    o[:, :, 4:6].rearrange("p j k -> p k j"),
                    t45[:],
                    0.5,
                    0.5,
                    Op.mult,
                    Op.add,
                )
                if gate_stores:
                    gate_eng.tensor_scalar(
                        o[:, 0:1, 0], o[:, 0:1, 0], x_last[:, 0, 0:1], None, Op.bypass
                    )
                nc.gpsimd.dma_start(
                    out=out_r[q * P : (q + 1) * P],
                    in_=o[:].rearrange("p j k -> p (j k)"),
                )
    nc.compile()
    return nc


def _consts() -> np.ndarray:
    """[128, 87] f32 constant table, loaded once into SBUF.

    Grid addends exploit (sigmoid(v) + g)*2 == tanh(v/2) + (2g + 1) and the
    tiling s = p*864 + jj*48 + x (so x = s%48 depends only on the inner free
    index, while y/z depend only on (p, jj)); they are read through stride-0
    broadcast APs instead of materializing the full [3, S] grid.

      [:, 0:48]   2*x + 1        (same for every partition)
      [:, 48:66]  2*y + 1        per (p, jj)
      [:, 66:84]  2*z + 1        per (p, jj)
      [:, 84:87]  ln(anchor_w)
    """
    t = np.empty((P, 48 + YZ + YZ + A), dtype=np.float32)
    x = np.arange(G, dtype=np.float32)
    yz = np.arange(P, dtype=np.int64)[:, None] * YZ + np.arange(YZ)[None, :]
    t[:, 0:G] = x * STRIDE + 1.0
    t[:, G : G + YZ] = (yz % G) * STRIDE + 1.0
    t[:, G + YZ : G + 2 * YZ] = (yz // G) * STRIDE + 1.0
    t[:, G + 2 * YZ :] = np.log(np.array(ANCHOR_W, dtype=np.float32))
    return t


def _build(
    split: int = 1,
    store_engine: str = "gpsimd",
    load_engine: str = "sync",
    per_channel_loads: bool = False,
    io_bufs: int = 5,
    out_bufs: int | None = 4,
    tmp_bufs: int = 4,
    sig_engine: str = "vector",
    exp_copy: bool = False,
    gate_stores: bool = True,
    gate_at_end: bool = False,
    gate_lead: int = 2,
    scalar_ring_loads: int = 0,
) -> bass.Bass:
    """Build the Bass program.

    Loads are issued from the Sync engine (HWDGE ring) and stores from the
    GpSimd engine (SWDGE ring). Separate rings matter: stores are gated on
    compute semaphores, and on a shared FIFO ring a waiting store blocks
    later loads from reaching the wire, serializing reads after writes and
    losing the read/write overlap HBM can sustain (~15us on this kernel).

    split: sub-tiles per (b, a) block along the free (spatial) dim.
    """
    splits = split if isinstance(split, (list, tuple)) else [split] * (B_LOC * A)
    assert len(splits) == B_LOC * A
    for s_ in splits:
        assert FREE % s_ == 0 and (FREE // s_) % G == 0

    nc = bacc.Bacc("TRN2", target_bir_lowering=False, debug=False)
    inp = nc.dram_tensor(
        "inp", [B_LOC, A * ATTRS, G, G, G], mybir.dt.float32, kind="ExternalInput"
    )
    consts = nc.dram_tensor(
        "consts", [P, G + 2 * YZ + A], mybir.dt.float32, kind="ExternalInput"
    )
    out = nc.dram_tensor(
        "out", [B_LOC, A * S, ATTRS], mybir.dt.float32, kind="ExternalOutput"
    )

    inp_r = inp.ap().rearrange("b (a c) d h w -> (b a) c (d h w)", a=A)
    out_r = out.ap().rearrange("b (a p j) k -> (b a) p (j k)", a=A, p=P)

    F = mybir.ActivationFunctionType
    Op = mybir.AluOpType
    f32 = mybir.dt.float32

    ld = getattr(nc, load_engine)
    st = getattr(nc, store_engine)

    with TileContext(nc) as tc:
        with (
            tc.tile_pool(name="const", bufs=1) as cpool,
            tc.tile_pool(name="io", bufs=io_bufs) as iopool,
            tc.tile_pool(name="io_out", bufs=out_bufs or io_bufs) as opool,
            tc.tile_pool(name="tmp", bufs=tmp_bufs) as tpool,
        ):
            ct = cpool.tile([P, G + 2 * YZ + A], f32)
            # consts ride the scalar HWDGE ring (idle at this point) so the
            # first block load is the first trigger on the sync ring
            nc.scalar.dma_start(out=ct[:], in_=consts.ap())
            lw = ct[:, G + 2 * YZ :]
            sig_eng = getattr(nc, sig_engine)

            # Issue ALL loads up front on the load ring. With gate_stores the
            # store DMAs are data-gated on the last load (see below), so the
            # loads stream the full input at the engine-side load ceiling
            # (~413 GB/s) with zero store interference, then the stores blast
            # at the fabric peak. Without the gate the DMA engines' per-
            # descriptor round-robin favors the 20.7KB store descriptors 6:1
            # over the 3.4KB load descriptors, starving mid-kernel loads and
            # stalling compute (measured: block loads triggered at 9us not
            # completing until 42-73us).
            xs = []
            for blk in range(B_LOC * A):
                spl = splits[blk]
                FR = FREE // spl
                blk_in = inp_r[blk].rearrange("c (p u j) -> u p c j", p=P, u=spl)
                # First scalar_ring_loads blocks load via the scalar HWDGE
                # ring (issued before any ACT compute, so no credit stalls);
                # each DMA engine then has TWO HW queues to round-robin,
                # letting it pipeline descriptor processing of one against
                # data movement of the other.
                bld = nc.scalar if blk < scalar_ring_loads else ld
                for u in range(spl):
                    x = iopool.tile([P, ATTRS, FR], f32, tag="in")
                    if per_channel_loads:
                        for c in range(ATTRS):
                            bld.dma_start(out=x[:, c, :], in_=blk_in[u, :, c, :])
                    else:
                        bld.dma_start(out=x[:], in_=blk_in[u])
                    xs.append(x)
            x_last = xs[-1]

            ti = 0
            pend = []
            for blk in range(B_LOC * A):
                a = blk % A
                spl = splits[blk]
                FR = FREE // spl  # spatial positions per partition per sub-tile
                YZR = FR // G  # (y,z)-rows per partition per sub-tile
                for u in range(spl):
                    # grid addends as [P, YZR, G] stride-0 broadcast views:
                    # x varies along the inner free axis only, y/z vary per
                    # (partition, yz-row) only
                    grids = (
                        ct[:, 0:G].unsqueeze(1).broadcast_to([P, YZR, G]),
                        ct[:, G + u * YZR : G + (u + 1) * YZR]
                        .unsqueeze(2)
                        .broadcast_to([P, YZR, G]),
                        ct[:, G + YZ + u * YZR : G + YZ + (u + 1) * YZR]
                        .unsqueeze(2)
                        .broadcast_to([P, YZR, G]),
                    )
                    x = xs[ti]
                    ti += 1
                    o = opool.tile([P, FR, ATTRS], f32, tag="out")
                    # All ACT ops are tanh/exp -> single exp_and_others table
                    # set for the whole kernel (sigmoid would force ~2.7us
                    # table reloads per block):
                    #   channels 0..2: sigmoid(v)*2 + 2g == tanh(v/2) + (2g+1)
                    #   channels 4,5:  sigmoid(v) == 0.5*tanh(v/2) + 0.5
                    for c in range(3):
                        t = tpool.tile([P, FR], f32, tag="t")
                        nc.scalar.activation(t[:], x[:, c, :], F.Tanh, scale=0.5)
                        nc.vector.tensor_add(
                            o[:, :, c].rearrange("p (r g) -> p r g", g=G),
                            t[:].rearrange("p (r g) -> p r g", g=G),
                            grids[c],
                        )
                    # channel 3: exp(v) * anchor_w[a] == exp(v + ln(anchor_w[a]))
                    if exp_copy:
                        # ACT pays 1.8x for strided writes; write unit-stride
                        # and let the otherwise-idle GpSimd do the interleave.
                        te = tpool.tile([P, FR], f32, tag="t")
                        nc.scalar.activation(
                            te[:], x[:, 3, :], F.Exp, bias=lw[:, a : a + 1]
                        )
                        nc.gpsimd.tensor_copy(o[:, :, 3], te[:])
                    else:
                        nc.scalar.activation(
                            o[:, :, 3], x[:, 3, :], F.Exp, bias=lw[:, a : a + 1]
                        )
                    for c in (4, 5):
                        t = tpool.tile([P, FR], f32, tag="t")
                        nc.scalar.activation(t[:], x[:, c, :], F.Tanh, scale=0.5)
                        sig_eng.tensor_scalar(
                            o[:, :, c], t[:], 0.5, 0.5, Op.mult, Op.add
                        )
                    if gate_stores and not gate_at_end:
                        # Benign self-copy of one element of o that also reads
                        # a LATER block's input tile: a real data dependency
                        # that keeps the loads at least `gate_lead` blocks
                        # ahead of store release. The DMA engines round-robin
                        # per DESCRIPTOR between their load and store queues,
                        # and store descriptors are 6x larger, so unthrottled
                        # stores starve pending loads ~6:1 and stall compute;
                        # this gate self-balances: when loads flow freely it
                        # costs nothing, when they lag it pauses new stores.
                        xg = xs[min(ti - 1 + gate_lead, len(xs) - 1)]
                        nc.gpsimd.tensor_scalar(
                            o[:, 0:1, 0], o[:, 0:1, 0], xg[:, 0, 0:1], None, Op.bypass
                        )
                        st.dma_start(
                            out=out_r[blk][:, u * FR * ATTRS : (u + 1) * FR * ATTRS],
                            in_=o[:].rearrange("p j k -> p (j k)"),
                        )
                    else:
                        pend.append((o, blk, u, FR))
            for o, blk, u, FR in pend:
                if gate_stores:
                    # alternate gate engines so the chain of tiny gate ops
                    # isn't serialized on one engine
                    geng = nc.gpsimd if (blk % 2 == 0) else nc.vector
                    geng.tensor_scalar(
                        o[:, 0:1, 0], o[:, 0:1, 0], x_last[:, 0, 0:1], None, Op.bypass
                    )
                st.dma_start(
                    out=out_r[blk][:, u * FR * ATTRS : (u + 1) * FR * ATTRS],
                    in_=o[:].rearrange("p j k -> p (j k)"),
                )
    nc.compile()
    return nc


_CONSTS_FN = None


def kernel(inp: np.ndarray) -> np.ndarray:
    global _NC, last_results, _CONSTS_FN
    if _NC is None:
        import json
        import os

        cfg = json.loads(os.environ.get("KCFG", "{}"))
        if cfg.pop("pairs", False):
            _NC = _build_pairs(**cfg)
            _CONSTS_FN = _consts2
        else:
            cfg.pop("legacy", None)
            _NC = _build(**cfg)
            _CONSTS_FN = _consts
    consts = _CONSTS_FN()
    inp = np.ascontiguousarray(np.asarray(inp), dtype=np.float32)
    assert inp.shape == (B, A * ATTRS, G, G, G), inp.shape
    in_maps = [
        {"inp": inp[i * B_LOC : (i + 1) * B_LOC], "consts": consts}
        for i in range(N_CORES)
    ]
    last_results = run_bass_kernel_spmd(
        _NC, in_maps, core_ids=list(range(N_CORES)), trace=trace
    )
    return np.concatenate([r["out"] for r in last_results.results], axis=0)

